# revision 1
# baseline (speedup 1.0000x reference)
"""Bahdanau-attention scores kernel for Trainium2, 8-core data-parallel.

Computes softmax_s( v . tanh(W_h @ h[b] + W_e @ enc[s,b] + bias) ) for
B=32, S=2048, Dd=512, De2=1024, sharded 4 batches per NeuronCore.

Per-core device layout (host pre-shards / pre-tiles into per-partition
form so every DMA is 128 long contiguous runs; r = b_local*2048 + s):
  encB      [128, 8*8*1024] fp16 encB[p,(t,k,r)] = enc^T[128k+p, 1024t+r]
  enc_first [128, 8*512]  fp16  block-0 first half, pre-tiled contiguous
  w_eT      [128, 4*8*128] fp16 w_eT[p, (j,k,oo)] = W_e[128j+oo, 128k+p]
  hb_in     [128, 4*4]    f32   hb_in[p,(j,b)] = (hidden @ W_h^T + bias)[b, 128j+p]
  v_pb      [128, 4]      f32   v_pb[p, j] = v[128j + p]
  v_pb16    [128, 4]      fp16  same, fp16 (final-block PE v-dot)
Output:
  probs     [4, 2048]     f32

The h-projection (hidden @ W_h^T + bias, 0.02% of total FLOPs) is
precomputed on host in exact f32 and shipped as a per-partition bias
table; everything else runs on device:
  E^T[o, r]  = sum_k W_e^T[k, o] encT[k, r]            (PE fp16, 8 k-chunks)
  et[o, r]   = tanh(E^T + hb[:, b])                    (ACT, per-partition bias)
  prod[o, r] = et * v[o]  summed over 4 o-chunks       (DVE mul/add tree, fp16)
  sc[r]      = ones^T @ prod                           (PE, K=128 -> [1, 512])
  expo       = exp(sc - 20), partial sums via accum_out (ACT, streaming softmax)
  probs[b,:] = expo / sum(expo)                        (DVE, per-batch finalize)

A run of warm-up matmuls on a memset tile covers the initial DMA window
so the PE HAM clock-gate is released (2.4 GHz) before the real stream
begins. DMA issue order is arranged so enc blocks are never queued
behind small transfers.
"""

import numpy as np

B = 32
S = 2048
DD = 512
DE2 = 1024
NCORES = 8
BL = B // NCORES  # 4 batches per core
R = BL * S  # 8192 rows per core
NK = DE2 // 128  # 8 k-chunks
NO = DD // 128  # 4 o-chunks
NB2 = R // 1024  # 8 DMA blocks of 1024 rows
EXP_OFF = -20.0  # softmax shift; scores observed in [-32, 27]
NWARM = 88

_CACHE = {}


def _build_bass():
    import concourse.bacc as bacc
    import concourse.mybir as mybir
    import concourse.tile as tile
    from concourse._compat import get_trn_type

    f32 = mybir.dt.float32
    f16 = mybir.dt.float16
    AF = mybir.ActivationFunctionType

    nc = bacc.Bacc(get_trn_type() or "TRN2", target_bir_lowering=False, debug=False)

    encB = nc.dram_tensor("encB", [128, NB2 * NK * 1024], f16, kind="ExternalInput")
    enc_first = nc.dram_tensor("enc_first", [128, NK * 512], f16, kind="ExternalInput")
    w_eT = nc.dram_tensor("w_eT", [128, NO * NK * 128], f16, kind="ExternalInput")
    hb_in = nc.dram_tensor("hb_in", [128, NO * BL], f32, kind="ExternalInput")
    v_pb = nc.dram_tensor("v_pb", [128, NO], f32, kind="ExternalInput")
    v_pb16 = nc.dram_tensor("v_pb16", [128, NO], f16, kind="ExternalInput")
    probs = nc.dram_tensor("probs", [BL, S], f32, kind="ExternalOutput")

    with tile.TileContext(nc) as tc:
        with (
            tc.tile_pool(name="const", bufs=1) as const,
            tc.tile_pool(name="encp", bufs=6) as encp,
            tc.tile_pool(name="etp", bufs=8) as etp,
            tc.tile_pool(name="prp", bufs=12) as prp,
            tc.tile_pool(name="pep", bufs=4, space="PSUM") as pep,
            tc.tile_pool(name="pmisc", bufs=3, space="PSUM") as pmisc,
            tc.tile_pool(name="pwu", bufs=1, space="PSUM") as pwu,
        ):
            # ---- PE warm-up: dummy matmuls while DMAs stream in ----
            warm_sb = const.tile([128, 128], f16, name="warm_sb")
            nc.any.memset(warm_sb[:], 0.0)
            wu_ps = pwu.tile([128, 128], f32, name="wu_ps", tag="wu")
            for i in range(NWARM):
                nc.tensor.matmul(
                    wu_ps[:], warm_sb[:], warm_sb[:], start=True, stop=True
                )

            # ---- critical-path DMAs, dual-issue: enc stream on Sync,
            # ---- weights/bias channel on Scalar (also HWDGE) ----
            encB_v = encB[:].rearrange("p (t k r) -> p t k r", t=NB2, k=NK)
            enc_first_v = enc_first[:].rearrange("p (k r) -> p k r", k=NK)
            b0h0 = const.tile([128, NK, 512], f16, name="b0h0")
            nc.sync.dma_start(b0h0[:], enc_first_v[:])
            # we_sb[p, j, k, oo] = W_e[128j+oo, 128k+p]; per-j DMAs so the
            # first matmul group only waits on 256 KB of weights
            we_sb = const.tile([128, NO, NK, 128], f16, name="we_sb")
            we_v = w_eT[:].rearrange("p (j k oo) -> p j k oo", j=NO, k=NK)
            for j in range(NO):
                nc.scalar.dma_start(we_sb[:, j], we_v[:, j])
            hb_sb = const.tile([128, NO, BL], f32, name="hb_sb")
            nc.scalar.dma_start(hb_sb[:], hb_in[:].rearrange("p (j b) -> p j b", j=NO))
            v_sb = const.tile([128, NO], f32, name="v_sb")
            nc.scalar.dma_start(v_sb[:], v_pb[:])
            v16_sb = const.tile([128, NO], f16, name="v16_sb")
            nc.scalar.dma_start(v16_sb[:], v_pb16[:])
            b0h1 = const.tile([128, NK, 512], f16, name="b0h1")
            nc.sync.dma_start(b0h1[:], encB_v[:, 0, :, 512:1024])

            ones_v = const.tile([128, 1], f16, name="ones_v")
            nc.any.memset(ones_v[:], 1.0)
            expoff_sb = const.tile([1, 1], f32, name="expoff_sb")
            nc.any.memset(expoff_sb[:], EXP_OFF)
            expo_flat = const.tile([1, R], f32, name="expo_flat")
            sumparts = const.tile([1, 4 * BL], f32, name="sumparts")
            outp = const.tile([1, R], f32, name="outp")

            def emit_exp(sc, t_i):
                # streaming softmax numerator + partial sum
                nc.scalar.activation(
                    expo_flat[0:1, 512 * t_i : 512 * (t_i + 1)],
                    sc[:],
                    AF.Exp,
                    bias=expoff_sb[:],
                    accum_out=sumparts[0:1, t_i : t_i + 1],
                )

            def emit_finalize(b):
                rsum = const.tile([1, 1], f32, name=f"rsum{b}", tag=f"rs{b}")
                nc.vector.reduce_sum(
                    rsum[:],
                    sumparts[0:1, 4 * b : 4 * (b + 1)],
                    axis=mybir.AxisListType.X,
                )
                rec = const.tile([1, 1], f32, name=f"rec{b}", tag=f"rc{b}")
                nc.vector.reciprocal(rec[:], rsum[:])
                # halves: first store overlaps the second half's multiply
                for u in range(2):
                    lo2 = S * b + (S // 2) * u
                    hi2 = lo2 + S // 2
                    nc.vector.tensor_scalar_mul(
                        outp[0:1, lo2:hi2], expo_flat[0:1, lo2:hi2], rec[:]
                    )
                    nc.scalar.dma_start(
                        probs[b : b + 1, (S // 2) * u : (S // 2) * (u + 1)],
                        outp[0:1, lo2:hi2],
                    )

            def emit_score(st):
                # ones-matmul deferred one half-block: its DVE-tree input is
                # long finished, so the PE never stalls on sem 157
                p0, b0_, t0_ = st
                sc = pmisc.tile([1, 512], f32, name="sc", tag="mi")
                nc.tensor.matmul(sc[:], ones_v[:], p0[:], start=True, stop=True)
                emit_exp(sc, t0_)
                if t0_ % 4 == 3:
                    emit_finalize(b0_)

            # ---- main loop: 8 DMA blocks x 2 halves of 512 rows ----
            pending = []
            for t2 in range(NB2):
                if t2 == 0:
                    halves = [b0h0, b0h1]
                else:
                    enc_t = encp.tile([128, NK, 1024], f16, name="enc_t", tag="enc")
                    nc.sync.dma_start(enc_t[:], encB_v[:, t2])
                    halves = [enc_t, enc_t]
                b = t2 // 2
                for h in range(2):
                    t_i = 2 * t2 + h  # 512-row block index, 4 per batch
                    last = t_i == 2 * NB2 - 1
                    src = halves[h]
                    lo = 0 if t2 == 0 else 512 * h
                    if len(pending) >= 3:
                        # flush three deferred scores adjacently: one
                        # weight-switch in/out per trio instead of per matmul;
                        # every entry is at least a half-block old so the PE
                        # never waits on its DVE tree
                        for st in pending:
                            emit_score(st)
                        pending = []
                    et_list = []
                    prods = []
                    for j in range(NO):
                        pe = pep.tile([128, 512], f32, name="pe", tag="pe")
                        for k in range(NK):
                            nc.tensor.matmul(
                                pe[:],
                                we_sb[:, j, k, :],
                                src[:, k, lo : lo + 512],
                                start=(k == 0),
                                stop=(k == NK - 1),
                            )
                        et = etp.tile([128, 512], f16, name="et", tag="et")
                        nc.scalar.activation(
                            et[:], pe[:], AF.Tanh, bias=hb_sb[:, j, b : b + 1]
                        )
                        et_list.append(et)
                        if not last:
                            # v-scale immediately so the DVE tree tracks the tanhs
                            pj = prp.tile([128, 512], f16, name=f"pj{j}", tag="pr")
                            nc.vector.tensor_scalar_mul(
                                pj[:], et[:], v_sb[:, j : j + 1]
                            )
                            prods.append(pj)
                    if last:
                        # final block: PE v-dot directly (shortest dep chain),
                        # after flushing this block's other deferred score
                        for st in pending:
                            emit_score(st)
                        pending = []
                        sc = pmisc.tile([1, 512], f32, name="sc", tag="mi")
                        for j in range(NO):
                            nc.tensor.matmul(
                                sc[:],
                                v16_sb[:, j : j + 1],
                                et_list[j][:],
                                start=(j == 0),
                                stop=(j == NO - 1),
                            )
                        emit_exp(sc, t_i)
                        emit_finalize(b)
                    else:
                        nc.vector.tensor_add(prods[0][:], prods[0][:], prods[1][:])
                        nc.vector.tensor_add(prods[2][:], prods[2][:], prods[3][:])
                        nc.vector.tensor_add(prods[0][:], prods[0][:], prods[2][:])
                        pending.append((prods[0], b, t_i))

    nc.compile()
    return nc


def _get_nc():
    if "nc" not in _CACHE:
        _CACHE["nc"] = _build_bass()
    return _CACHE["nc"]


def _tile_rows(mat_t, nchunk):
    # [nchunk*128, F] -> [128, nchunk*F] with out[p, c*F+f] = mat_t[128c+p, f]
    n, F = mat_t.shape
    assert n == nchunk * 128
    return np.ascontiguousarray(
        mat_t.reshape(nchunk, 128, F).transpose(1, 0, 2)
    ).reshape(128, nchunk * F)


def _make_in_maps(hidden, enc, W, b, v):
    W_h = W[:, :DD]
    W_e = W[:, DD:]
    # w_eT[p, j, k, oo] = W_e[128j+oo, 128k+p]
    w_eT = np.ascontiguousarray(
        W_e.reshape(NO, 128, NK, 128).transpose(3, 0, 2, 1)
    ).reshape(128, NO * NK * 128).astype(np.float16)
    v_pb = np.ascontiguousarray(v.reshape(NO, 128).T).astype(np.float32)
    v_pb16 = v_pb.astype(np.float16)
    enc16 = enc.astype(np.float16)  # [S, B, DE2]
    in_maps = []
    for c in range(NCORES):
        ec = enc16[:, BL * c : BL * (c + 1), :]  # [S, BL, DE2]
        encT = np.ascontiguousarray(ec.transpose(2, 1, 0)).reshape(DE2, R)
        # encB[p, t2, k, r] = encT[128k+p, 1024*t2 + r] (contiguous per block)
        encB = np.ascontiguousarray(
            encT.reshape(NK, 128, NB2, 1024).transpose(1, 2, 0, 3)
        ).reshape(128, NB2 * NK * 1024)
        enc_first = _tile_rows(np.ascontiguousarray(encT[:, :512]), NK)
        # exact f32 h-projection + bias, tiled per-partition: [128, (j, b)]
        h_proj = hidden[BL * c : BL * (c + 1), :] @ W_h.T + b  # [BL, DD]
        hb = _tile_rows(np.ascontiguousarray(h_proj.T), NO)  # [128, NO*BL]
        in_maps.append(
            {
                "encB": encB,
                "enc_first": enc_first,
                "w_eT": w_eT,
                "hb_in": np.ascontiguousarray(hb, dtype=np.float32),
                "v_pb": v_pb,
                "v_pb16": v_pb16,
            }
        )
    return in_maps


def kernel(hidden, encoder_outputs, W, b, v):
    """Full inputs in, full output out; 8-way batch-parallel inside."""
    from concourse.bass_utils import run_bass_kernel_spmd

    hidden = np.asarray(hidden, dtype=np.float32)
    enc = np.asarray(encoder_outputs, dtype=np.float32)
    W = np.asarray(W, dtype=np.float32)
    b = np.asarray(b, dtype=np.float32)
    v = np.asarray(v, dtype=np.float32)

    in_maps = _make_in_maps(hidden, enc, W, b, v)
    nc = _get_nc()
    res = run_bass_kernel_spmd(nc, in_maps, core_ids=list(range(NCORES)))
    out = np.concatenate([res.results[c]["probs"] for c in range(NCORES)], axis=0)
    return out.astype(np.float32)



# revision 9
# speedup vs baseline: 1.2172x; 1.2172x over previous
"""Bahdanau-attention scores kernel for Trainium2, 8-core data-parallel.

Computes softmax_s( v . tanh(W_h @ h[b] + W_e @ enc[s,b] + bias) ) for
B=32, S=2048, Dd=512, De2=1024, sharded 4 batches per NeuronCore.

Two-precision scheme (single launch):
  Pass 1 (fp8):  E^T = W_e8 @ enc8 on the PE in e4m3 DoubleRow mode
                 (K=256 per pass, 2x fp16 rate). tanh on ACT with
                 scale=1/128 dequant + per-partition h-projection bias,
                 v-weighted sum via DVE tree, scores via ones-matmul.
  Select:        per batch row, scores land as [16,128] (DRAM-roundtrip
                 relayout); top-8 of each 128-chunk via max_with_indices
                 = 128 candidates/row. fp8 score error (~0.2) only
                 matters for positions near the row max; top-8/chunk
                 covers everything with softmax weight > ~e^-6.
  Refine (fp16): gather the 128 selected enc rows (indirect DMA), PE
                 transpose, recompute scores in fp16, exp, and merge
                 back into the fp8 exp row via 8 predicated copies.
  Finalize:      row sum of merged exp -> reciprocal -> scale -> DMA.

The h-projection (hidden @ W_h^T + bias) is precomputed on host in
exact f32 and shipped as a per-partition bias table.
"""

import numpy as np

B = 32
S = 2048
DD = 512
DE2 = 1024
NCORES = 8
BL = B // NCORES  # 4 batches per core
R = BL * S  # 8192 rows per core
NK = DE2 // 128  # 8 k-chunks
NO = DD // 128  # 4 o-chunks
NB2 = R // 1024  # 8 DMA blocks of 1024 rows
EXP_OFF = -26.0  # softmax shift; scores observed in [-32, 27]
W8SCALE = 128.0  # fp8 weight pre-scale (keeps W_e out of e4m3 subnormals)
NWARM = 88

_CACHE = {}


def _build_bass():
    import concourse.bacc as bacc
    import concourse.mybir as mybir
    import concourse.tile as tile
    import concourse.bass as bass
    from concourse._compat import get_trn_type

    f32 = mybir.dt.float32
    f16 = mybir.dt.float16
    f8 = mybir.dt.float8e4
    i32 = mybir.dt.int32
    u32 = mybir.dt.uint32
    AF = mybir.ActivationFunctionType
    DR = mybir.MatmulPerfMode.DoubleRow

    nc = bacc.Bacc(get_trn_type() or "TRN2", target_bir_lowering=False, debug=False)

    encB8 = nc.dram_tensor("encB8", [128, NB2 * NK * 1024], f8, kind="ExternalInput")
    w8 = nc.dram_tensor("w8", [128, NO * NK * 128], f8, kind="ExternalInput")
    w16 = nc.dram_tensor("w16", [128, NO * NK * 128], f16, kind="ExternalInput")
    hb_in = nc.dram_tensor("hb_in", [128, NO * BL], f32, kind="ExternalInput")
    v_pb = nc.dram_tensor("v_pb", [128, NO], f32, kind="ExternalInput")
    encP16 = nc.dram_tensor("encP16", [R, DE2], f16, kind="ExternalInput")
    ident16 = nc.dram_tensor("ident16", [128, 128], f16, kind="ExternalInput")
    iotac_in = nc.dram_tensor("iotac_in", [16, 128], f32, kind="ExternalInput")
    posb_in = nc.dram_tensor("posb_in", [16, BL], f32, kind="ExternalInput")
    probs = nc.dram_tensor("probs", [BL, S], f32, kind="ExternalOutput")
    scr_dram = nc.dram_tensor("scr_dram", [BL, S], f32, kind="ExternalOutput")
    scl_dram = nc.dram_tensor("scl_dram", [BL, 128], f32, kind="ExternalOutput")
    idx_dram = nc.dram_tensor("idx_dram", [BL, 128], i32, kind="ExternalOutput")

    with tile.TileContext(nc) as tc:
        with (
            tc.tile_pool(name="const", bufs=1) as const,
            tc.tile_pool(name="encp", bufs=3) as encp,
            tc.tile_pool(name="etp", bufs=6) as etp,
            tc.tile_pool(name="prp", bufs=6) as prp,
            tc.tile_pool(name="refp", bufs=2) as refp,
            tc.tile_pool(name="pep", bufs=4, space="PSUM") as pep,
            tc.tile_pool(name="pmisc", bufs=2, space="PSUM") as pmisc,
            tc.tile_pool(name="pref", bufs=2, space="PSUM") as pref,
        ):
            # ---- PE warm-up: dummy matmuls while DMAs stream in ----
            warm_sb = const.tile([128, 128], f16, name="warm_sb")
            nc.any.memset(warm_sb[:], 0.0)
            wu_ps = pmisc.tile([128, 128], f32, name="wu_ps", tag="mi")
            for i in range(NWARM):
                nc.tensor.matmul(
                    wu_ps[:], warm_sb[:], warm_sb[:], start=True, stop=True
                )

            # ---- constants / weights (scalar queue) ----
            encB_v = encB8[:].rearrange("p (t k r) -> p t k r", t=NB2, k=NK)
            w8_sb = const.tile([128, NO, NK, 128], f8, name="w8_sb")
            w8_v = w8[:].rearrange("p (j k oo) -> p j k oo", j=NO, k=NK)
            for j in range(NO):
                nc.scalar.dma_start(w8_sb[:, j], w8_v[:, j])
            hb_sb = const.tile([128, NO, BL], f32, name="hb_sb")
            nc.scalar.dma_start(hb_sb[:], hb_in[:].rearrange("p (j b) -> p j b", j=NO))
            v_sb = const.tile([128, NO], f32, name="v_sb")
            nc.scalar.dma_start(v_sb[:], v_pb[:])
            w16_sb = const.tile([128, NO, NK, 128], f16, name="w16_sb")
            nc.scalar.dma_start(
                w16_sb[:], w16[:].rearrange("p (j k oo) -> p j k oo", j=NO, k=NK)
            )
            id_sb = const.tile([128, 128], f16, name="id_sb")
            nc.scalar.dma_start(id_sb[:], ident16[:])
            iotac = const.tile([16, 128], f32, name="iotac")
            nc.scalar.dma_start(iotac[:], iotac_in[:])
            posb = const.tile([16, BL], f32, name="posb")
            nc.scalar.dma_start(posb[:], posb_in[:])

            ones_v = const.tile([128, 1], f16, name="ones_v")
            nc.any.memset(ones_v[:], 1.0)
            ones16 = const.tile([16, 1], f32, name="ones16")
            nc.any.memset(ones16[:], 1.0)
            onesb = const.tile([1, 16], f32, name="onesb")
            nc.any.memset(onesb[:], 1.0)
            expoff16 = const.tile([16, 1], f32, name="expoff16")
            nc.any.memset(expoff16[:], EXP_OFF)
            scrow = [
                const.tile([1, S], f32, name=f"scrow{b}") for b in range(2)
            ]  # double-buffered per-row score rows

            def emit_score(st):
                p0, b0_, t0_ = st
                sc = pmisc.tile([1, 512], f32, name="sc", tag="mi")
                nc.tensor.matmul(sc[:], ones_v[:], p0[:], start=True, stop=True)
                nc.scalar.copy(scrow[b0_ % 2][0:1, 512 * t0_ : 512 * (t0_ + 1)], sc[:])

            def emit_refine(b):
                row = scrow[b % 2]
                # relayout scores [1,2048] -> [16,128] via DRAM roundtrip
                nc.gpsimd.dma_start(scr_dram[b : b + 1], row[:])
                sc16 = refp.tile([16, 128], f32, name="sc16", tag="sc16")
                nc.gpsimd.dma_start(
                    sc16[:], scr_dram[b].rearrange("(p t) -> p t", p=16)
                )
                expo16 = refp.tile([16, 128], f32, name="expo16", tag="ex16")
                nc.scalar.activation(expo16[:], sc16[:], AF.Exp, bias=expoff16[:])
                # top-8 per 128-chunk
                m8 = refp.tile([16, 8], f32, name="m8", tag="m8")
                mi = refp.tile([16, 8], u32, name="mi", tag="mi8")
                nc.vector.max_with_indices(m8[:], mi[:], sc16[:])
                mif = refp.tile([16, 8], f32, name="mif", tag="mif")
                nc.vector.tensor_copy(mif[:], mi[:])
                idxgf = refp.tile([16, 8], f32, name="idxgf", tag="idxgf")
                nc.vector.tensor_scalar(
                    idxgf[:], mif[:], posb[:, b : b + 1], None, mybir.AluOpType.add
                )
                idxg = refp.tile([16, 8], i32, name="idxg", tag="idxg")
                nc.vector.tensor_copy(idxg[:], idxgf[:])
                # indices -> [128,1] via DRAM roundtrip, then gather enc rows
                nc.gpsimd.dma_start(
                    idx_dram[b].rearrange("(p j) -> p j", p=16), idxg[:]
                )
                idx128 = refp.tile([128, 1], i32, name="idx128", tag="i128")
                nc.gpsimd.dma_start(
                    idx128[:], idx_dram[b].rearrange("(p j) -> p j", p=128)
                )
                gath = refp.tile([128, DE2], f16, name="gath", tag="gath")
                nc.gpsimd.indirect_dma_start(
                    out=gath[:],
                    out_offset=None,
                    in_=encP16[:],
                    in_offset=bass.IndirectOffsetOnAxis(ap=idx128[:, :1], axis=0),
                )
                # PE transpose to [128 feat, 128 pos] per k-chunk
                encsel = refp.tile([128, NK, 128], f16, name="encsel", tag="esel")
                for k in range(NK):
                    tp = pref.tile([128, 128], f16, name="tp", tag="rf")
                    nc.tensor.transpose(
                        tp[:], gath[:, 128 * k : 128 * (k + 1)], id_sb[:]
                    )
                    nc.vector.tensor_copy(encsel[:, k], tp[:])
                # fp16 recompute of the 128 selected scores
                ret = []
                for j in range(NO):
                    rpe = pref.tile([128, 128], f32, name="rpe", tag="rf")
                    for k in range(NK):
                        nc.tensor.matmul(
                            rpe[:],
                            w16_sb[:, j, k, :],
                            encsel[:, k, :],
                            start=(k == 0),
                            stop=(k == NK - 1),
                        )
                    rt = refp.tile([128, 128], f16, name="rt", tag=f"rt{j}")
                    nc.scalar.activation(
                        rt[:], rpe[:], AF.Tanh, bias=hb_sb[:, j, b : b + 1]
                    )
                    ret.append(rt)
                scsel = pref.tile([1, 128], f32, name="scsel", tag="rf")
                for j in range(NO):
                    nc.tensor.matmul(
                        scsel[:],
                        v16_sb[:, j : j + 1],
                        ret[j][:],
                        start=(j == 0),
                        stop=(j == NO - 1),
                    )
                scselS = refp.tile([1, 128], f32, name="scselS", tag="sclS")
                nc.scalar.copy(scselS[:], scsel[:])
                # [1,128] -> [16,8] roundtrip, exp, merge into expo16
                nc.gpsimd.dma_start(scl_dram[b : b + 1], scselS[:])
                scs16 = refp.tile([16, 8], f32, name="scs16", tag="scs16")
                nc.gpsimd.dma_start(
                    scs16[:], scl_dram[b].rearrange("(p j) -> p j", p=16)
                )
                es16 = refp.tile([16, 8], f32, name="es16", tag="es16")
                nc.scalar.activation(es16[:], scs16[:], AF.Exp, bias=expoff16[:])
                for j in range(8):
                    mj = refp.tile([16, 128], mybir.dt.int32, name="mj", tag="mj")
                    nc.vector.tensor_scalar(
                        mj[:], iotac[:], mif[:, j : j + 1], None,
                        mybir.AluOpType.is_equal,
                    )
                    nc.vector.copy_predicated(
                        expo16[:], mj[:], es16[:, j : j + 1].to_broadcast([16, 128])
                    )
                # row sum -> reciprocal -> broadcast -> normalize -> out
                rsum = refp.tile([16, 1], f32, name="rsum", tag="rsum")
                nc.vector.reduce_sum(rsum[:], expo16[:], axis=mybir.AxisListType.X)
                tot = pref.tile([1, 1], f32, name="tot", tag="rf")
                nc.tensor.matmul(tot[:], ones16[:], rsum[:], start=True, stop=True)
                totS = refp.tile([1, 1], f32, name="totS", tag="totS")
                nc.vector.tensor_copy(totS[:], tot[:])
                rec = refp.tile([1, 1], f32, name="rec", tag="rec")
                nc.vector.reciprocal(rec[:], totS[:])
                recb = pref.tile([16, 1], f32, name="recb", tag="rf")
                nc.tensor.matmul(recb[:], onesb[:], rec[:], start=True, stop=True)
                recbS = refp.tile([16, 1], f32, name="recbS", tag="rcbS")
                nc.vector.tensor_copy(recbS[:], recb[:])
                probs16 = refp.tile([16, 128], f32, name="probs16", tag="p16")
                nc.vector.tensor_scalar_mul(probs16[:], expo16[:], recbS[:])
                nc.scalar.dma_start(
                    probs[b].rearrange("(p t) -> p t", p=16), probs16[:]
                )

            v16_sb = const.tile([128, NO], f16, name="v16_sb")
            nc.vector.tensor_copy(v16_sb[:], v_sb[:])

            # ---- main loop: 8 DMA blocks of 1024 rows (= half a batch) ----
            pending = []
            for t2 in range(NB2):
                enc_t = encp.tile([128, NK, 1024], f8, name="enc_t", tag="enc")
                nc.sync.dma_start(enc_t[:], encB_v[:, t2])
                b = t2 // 2
                # flush deferred score matmuls from the previous block
                for st in pending:
                    emit_score(st)
                pending = []
                if t2 >= 2 and t2 % 2 == 0:
                    emit_refine(t2 // 2 - 1)
                prodacc = [None, None]
                for j in range(NO):
                    pe_h = [
                        pep.tile([128, 512], f32, name="pe", tag="pe")
                        for _ in range(2)
                    ]
                    for kk in range(NK // 2):
                        for h in range(2):
                            nc.tensor.matmul(
                                pe_h[h][:],
                                w8_sb[:, j, 2 * kk : 2 * kk + 2, :],
                                enc_t[:, 2 * kk : 2 * kk + 2, 512 * h : 512 * (h + 1)],
                                start=(kk == 0),
                                stop=(kk == NK // 2 - 1),
                                perf_mode=DR,
                            )
                    for h in range(2):
                        et = etp.tile([128, 512], f16, name="et", tag="et")
                        nc.scalar.activation(
                            et[:],
                            pe_h[h][:],
                            AF.Tanh,
                            bias=hb_sb[:, j, b : b + 1],
                            scale=1.0 / W8SCALE,
                        )
                        if j == 0:
                            pa = prp.tile([128, 512], f16, name="pa", tag="pa")
                            nc.vector.tensor_scalar_mul(pa[:], et[:], v_sb[:, 0:1])
                            prodacc[h] = pa
                        else:
                            pj = prp.tile([128, 512], f16, name="pj", tag="pj")
                            nc.vector.tensor_scalar_mul(pj[:], et[:], v_sb[:, j : j + 1])
                            nc.vector.tensor_add(prodacc[h][:], prodacc[h][:], pj[:])
                for h in range(2):
                    t_i = (t2 % 2) * 2 + h
                    pending.append((prodacc[h], b, t_i))

            for st in pending:
                emit_score(st)
            emit_refine(BL - 1)

    nc.compile()
    return nc


def _get_nc():
    if "nc" not in _CACHE:
        _CACHE["nc"] = _build_bass()
    return _CACHE["nc"]


def _tile_rows(mat_t, nchunk):
    # [nchunk*128, F] -> [128, nchunk*F] with out[p, c*F+f] = mat_t[128c+p, f]
    n, F = mat_t.shape
    assert n == nchunk * 128
    return np.ascontiguousarray(
        mat_t.reshape(nchunk, 128, F).transpose(1, 0, 2)
    ).reshape(128, nchunk * F)


def _make_in_maps(hidden, enc, W, b, v):
    import ml_dtypes

    f8 = ml_dtypes.float8_e4m3
    W_h = W[:, :DD]
    W_e = W[:, DD:]
    # w[p, j, k, oo] = W_e[128j+oo, 128k+p]
    w_lay = np.ascontiguousarray(
        W_e.reshape(NO, 128, NK, 128).transpose(3, 0, 2, 1)
    ).reshape(128, NO * NK * 128)
    w8_arr = (w_lay * W8SCALE).astype(f8)
    w16_arr = w_lay.astype(np.float16)
    v_pb = np.ascontiguousarray(v.reshape(NO, 128).T).astype(np.float32)
    ident = np.eye(128, dtype=np.float16)
    iotac = np.broadcast_to(
        np.arange(128, dtype=np.float32), (16, 128)
    ).copy()
    posb = (
        2048.0 * np.arange(BL)[None, :] + 128.0 * np.arange(16)[:, None]
    ).astype(np.float32)
    in_maps = []
    for c in range(NCORES):
        ec = enc[:, BL * c : BL * (c + 1), :]  # [S, BL, DE2]
        encT = np.ascontiguousarray(ec.transpose(2, 1, 0)).reshape(DE2, R)
        encB = np.ascontiguousarray(
            encT.reshape(NK, 128, NB2, 1024).transpose(1, 2, 0, 3)
        ).reshape(128, NB2 * NK * 1024)
        encB8 = encB.astype(f8)
        encP16 = np.ascontiguousarray(ec.transpose(1, 0, 2)).reshape(R, DE2).astype(
            np.float16
        )
        # exact f32 h-projection + bias, tiled per-partition: [128, (j, b)]
        h_proj = hidden[BL * c : BL * (c + 1), :] @ W_h.T + b  # [BL, DD]
        hb = _tile_rows(np.ascontiguousarray(h_proj.T), NO)  # [128, NO*BL]
        in_maps.append(
            {
                "encB8": encB8,
                "w8": w8_arr,
                "w16": w16_arr,
                "hb_in": np.ascontiguousarray(hb, dtype=np.float32),
                "v_pb": v_pb,
                "encP16": encP16,
                "ident16": ident,
                "iotac_in": iotac,
                "posb_in": posb,
            }
        )
    return in_maps


def kernel(hidden, encoder_outputs, W, b, v):
    """Full inputs in, full output out; 8-way batch-parallel inside."""
    from concourse.bass_utils import run_bass_kernel_spmd

    hidden = np.asarray(hidden, dtype=np.float32)
    enc = np.asarray(encoder_outputs, dtype=np.float32)
    W = np.asarray(W, dtype=np.float32)
    b = np.asarray(b, dtype=np.float32)
    v = np.asarray(v, dtype=np.float32)

    in_maps = _make_in_maps(hidden, enc, W, b, v)
    nc = _get_nc()
    res = run_bass_kernel_spmd(nc, in_maps, core_ids=list(range(NCORES)))
    out = np.concatenate([res.results[c]["probs"] for c in range(NCORES)], axis=0)
    return out.astype(np.float32)
